# revision 8
# baseline (speedup 1.0000x reference)
"""MultiHeadCrossAttention TRN2 Bass kernel (8 NeuronCores, SPMD).

Problem (hardcoded): B=4, T_tgt=T_src=1024, EMB=1024, H=16 heads of 64.
  Q = x @ W_Q.T ; K = ctx @ W_K.T ; V = ctx @ W_V.T      (nn.Linear, bias=False)
  S = Q K^T / 8 ; S[mask] = -inf ; w = softmax(S); out = merge(w V) @ W_O.T
  reference returns (out, w)

Sharding: core c -> batch b = c//2, head-half = c%2 (8 of 16 heads).
Each core emits its heads' softmax weights [8,1024,1024] and a partial
output O^T [1024,1024] (contraction over its 512 emb dims); the host adds
core pairs and transposes.

Per-core dataflow (matmuls in fp32r = full PE rate):
  feature-major Q^T,K^T [d,(t|s)] via xT/cT streams; token-major V [s,d].
  layout A [t,s]: S_A (PSUM) -> exp (ACT) -> (exp*1)*m01 w/ accum row-sums
      (DVE scalar_tensor_tensor) -> *1/sum (DVE tensor_scalar) -> weights out
  layout B [s,t]: S_B += -32768*mask via bf16 identity-matmul -> exp (ACT,
      PSUM->SBUF) = w_B -> AV col-packed head pairs -> merged^T * (1/sum)
      broadcast rows (PE transpose + K=1 ones matmuls) -> merged^T
  O^T[j,t] = sum_d woT[d,j] merged^T[d,t]
"""

import numpy as np

import concourse.bass as bass
import concourse.mybir as mybir
import concourse.tile as tile
from concourse.bass_utils import run_bass_kernel_spmd
from concourse.masks import make_identity

FP = mybir.dt.float32
FR = mybir.dt.float32r
BF = mybir.dt.bfloat16

B, T, SRC, EMB, H = 4, 1024, 1024, 1024, 16
HPC = 8  # heads per core
HEAD = 64
D = HPC * HEAD  # 512 emb dims per core
SCALE = float(1.0 / np.sqrt(HEAD))
NEG = -32768.0
N_CORES = 8

AF = mybir.ActivationFunctionType
ALU = mybir.AluOpType

_wait_ctr = [0]


def _split_multi_waits(nc, cap=1):
    """This walrus build accepts one sync wait per engine instruction; move
    excess waits onto preceding InstEventSemaphore ops on the same engine."""
    for blk in nc.m.functions[0].blocks:
        insts = blk.instructions
        out = []
        changed = False
        for inst in insts:
            si = inst.sync_info
            waits = list(si.on_wait) if (si is not None and si.on_wait) else []
            if len(waits) > cap:
                for w in waits[:-cap]:
                    _wait_ctr[0] += 1
                    ev = mybir.InstEventSemaphore(
                        name=f"waitsplit_{_wait_ctr[0]}", ins=[], outs=[]
                    )
                    ev.engine = inst.engine
                    ev.sync_info = mybir.SyncInfo(on_wait=[w], on_update=[])
                    out.append(ev)
                inst.sync_info = mybir.SyncInfo(
                    on_wait=waits[-cap:],
                    on_update=list(si.on_update) if si.on_update else [],
                )
                changed = True
            out.append(inst)
        if changed:
            blk.instructions = out


def build_nc():
    nc = bass.Bass("TRN2", target_bir_lowering=False, debug=False)

    xT = nc.dram_tensor("xT", [EMB, T], FR, kind="ExternalInput").ap()
    cT = nc.dram_tensor("cT", [EMB, SRC], FR, kind="ExternalInput").ap()
    # packed [W_Q_part.T | W_K_part.T] ([EMB, 1024]) and W_V_part.T ([EMB, 512])
    wqk = nc.dram_tensor("wqk", [EMB, 2 * D], FR, kind="ExternalInput").ap()
    wv = nc.dram_tensor("wv", [EMB, D], FR, kind="ExternalInput").ap()
    woT = nc.dram_tensor("woT", [D, EMB], FR, kind="ExternalInput").ap()
    m01A = nc.dram_tensor("m01A", [T, SRC], BF, kind="ExternalInput").ap()
    mB = nc.dram_tensor("mB", [SRC, T], BF, kind="ExternalInput").ap()
    wout = nc.dram_tensor("wout", [HPC, T, SRC], FP, kind="ExternalOutput").ap()
    oout = nc.dram_tensor("oout", [EMB, T], FP, kind="ExternalOutput").ap()

    with tile.TileContext(nc) as tc:
        with (
            tc.tile_pool(name="const", bufs=1) as constp,
            tc.tile_pool(name="resident", bufs=1) as resp,
            tc.tile_pool(name="xs", bufs=3) as xsp,
            tc.tile_pool(name="wos", bufs=2) as wosp,
            tc.tile_pool(name="expA", bufs=2) as expAp,
            tc.tile_pool(name="wA", bufs=3) as wAp,
            tc.tile_pool(name="expB", bufs=3) as expBp,
            tc.tile_pool(name="osb", bufs=2) as osbp,
            tc.tile_pool(name="small", bufs=2) as smallp,
            tc.tile_pool(name="ps", bufs=1, space="PSUM") as psp,
        ):
            # ---- constants ----
            ident = constp.tile([128, 128], FP, name="ident")
            make_identity(nc, ident[:])
            identb = constp.tile([128, 128], BF, name="identb")
            make_identity(nc, identb[:])
            ones64 = constp.tile([128, 64], FP, name="ones64")
            nc.gpsimd.memset(ones64[:], 1.0)

            # ---- resident tiles ----
            wqk_s = resp.tile([128, 8, 2 * D], FR, name="wqk_s")
            nc.sync.dma_start(
                out=wqk_s[:], in_=wqk.rearrange("(c p) d -> p c d", p=128)
            )
            m01A_s = resp.tile([128, 8, SRC], BF, name="m01A_s")
            nc.sync.dma_start(
                out=m01A_s[:], in_=m01A.rearrange("(c p) s -> p c s", p=128)
            )
            mB_s = resp.tile([128, 8, T], BF, name="mB_s")
            nc.sync.dma_start(
                out=mB_s[:], in_=mB.rearrange("(c p) t -> p c t", p=128)
            )
            wv_s = resp.tile([128, 8, D], FR, name="wv_s")
            nc.sync.dma_start(
                out=wv_s[:], in_=wv.rearrange("(c p) d -> p c d", p=128)
            )
            qT_s = [resp.tile([128, T], FR, name=f"qT_{dc}") for dc in range(4)]
            kT_s = [resp.tile([128, SRC], FR, name=f"kT_{dc}") for dc in range(4)]
            v_s = [resp.tile([128, D], FR, name=f"v_{sc}") for sc in range(8)]
            merged_s = [
                resp.tile([128, T], FR, name=f"merged_{dc}") for dc in range(4)
            ]

            # ================= Phase 1a: Q^T projection =================
            # Q^T[dc][:, tc*512:] = sum_ec wq[ec,dc].T @ xT[ec, tc-half]
            for tc in range(2):
                psQ = [
                    psp.tile([128, 512], FP, name=f"psQ_{tc}_{dc}", tag="b1", bufs=4)
                    for dc in range(4)
                ]
                for ec in range(8):
                    xt = xsp.tile([128, 512], FR, name=f"xt_{tc}_{ec}", tag="xs")
                    nc.sync.dma_start(
                        out=xt[:],
                        in_=xT[ec * 128 : (ec + 1) * 128, tc * 512 : (tc + 1) * 512],
                    )
                    for dc in range(4):
                        nc.tensor.matmul(
                            psQ[dc][:],
                            (wqk_s[:, ec, dc * 128 : (dc + 1) * 128]),
                            (xt[:]),
                            start=(ec == 0),
                            stop=(ec == 7),
                        )
                for dc in range(4):
                    nc.scalar.copy(
                        qT_s[dc][:, tc * 512 : (tc + 1) * 512], psQ[dc][:]
                    )

            # ============ Phase 1b: K^T projection + V projection ============
            for hc in range(2):
                psK = [
                    psp.tile([128, 512], FP, name=f"psK_{hc}_{dc}", tag="b1", bufs=4)
                    for dc in range(4)
                ]
                psV = [
                    psp.tile([128, 1024], FP, name=f"psV_{hc}_{g}", tag="s", bufs=2)
                    for g in range(2)
                ]
                for ec in range(8):
                    ct = xsp.tile([128, 512], FR, name=f"ct_{hc}_{ec}", tag="xs")
                    nc.sync.dma_start(
                        out=ct[:],
                        in_=cT[ec * 128 : (ec + 1) * 128, hc * 512 : (hc + 1) * 512],
                    )
                    for dc in range(4):
                        nc.tensor.matmul(
                            psK[dc][:],
                            (wqk_s[:, ec, D + dc * 128 : D + (dc + 1) * 128]),
                            (ct[:]),
                            start=(ec == 0),
                            stop=(ec == 7),
                        )
                    for sv in range(4):
                        nc.tensor.matmul(
                            psV[sv // 2][:, (sv % 2) * 512 : (sv % 2 + 1) * 512],
                            (ct[:, sv * 128 : (sv + 1) * 128]),
                            (wv_s[:, ec, :]),
                            start=(ec == 0),
                            stop=(ec == 7),
                        )
                for dc in range(4):
                    nc.scalar.copy(
                        kT_s[dc][:, hc * 512 : (hc + 1) * 512], psK[dc][:]
                    )
                for sv in range(4):
                    nc.vector.tensor_copy(
                        v_s[hc * 4 + sv][:],
                        psV[sv // 2][:, (sv % 2) * 512 : (sv % 2 + 1) * 512],
                    )

            # ================= Phase 2: attention per head pair =================
            for dc in range(4):
                heads = (2 * dc, 2 * dc + 1)
                r_h = {
                    h: smallp.tile([128, 8], FP, name=f"r_{h}", tag="r", bufs=2)
                    for h in heads
                }

                # ---- B side: S_B + mask -> exp -> w_B -> AV ----
                av_ps = {
                    (hh, tc): psp.tile(
                        [64, 512], FP, name=f"av_{dc}_{hh}_{tc}", tag="b1", bufs=4
                    )
                    for hh in range(2)
                    for tc in range(2)
                }
                for sc in range(8):
                    sB = {}
                    for hh, h in enumerate(heads):
                        sB[h] = psp.tile(
                            [128, 1024], FP, name=f"sB_{h}_{sc}", tag="s", bufs=2
                        )
                    for th in range(2):
                        for hh, h in enumerate(heads):
                            nc.tensor.matmul(
                                sB[h][:, th * 512 : (th + 1) * 512],
                                (
                                    kT_s[dc][
                                        hh * 64 : hh * 64 + 64,
                                        sc * 128 : (sc + 1) * 128,
                                    ]
                                ),
                                (
                                    qT_s[dc][
                                        hh * 64 : hh * 64 + 64,
                                        th * 512 : (th + 1) * 512,
                                    ]
                                ),
                                start=True,
                                stop=False,
                                skip_group_check=True,
                            )
                    for th in range(2):
                        for hh, h in enumerate(heads):
                            nc.tensor.matmul(
                                sB[h][:, th * 512 : (th + 1) * 512],
                                identb[:],
                                mB_s[:, sc, th * 512 : (th + 1) * 512],
                                start=False,
                                stop=True,
                                skip_group_check=True,
                            )
                    for hh, h in enumerate(heads):
                        wB = expBp.tile(
                            [128, 1024], FR, name=f"wB_{h}_{sc}", tag="expB"
                        )
                        nc.scalar.activation(wB[:], sB[h][:], AF.Exp, scale=SCALE)
                        for tc in range(2):
                            nc.tensor.matmul(
                                av_ps[(hh, tc)][:],
                                (v_s[sc][:, h * 64 : (h + 1) * 64]),
                                (wB[:, tc * 512 : (tc + 1) * 512]),
                                start=(sc == 0),
                                stop=(sc == 7),
                            )

                # ---- A side: S_A -> exp -> mask+sums -> norm -> weights out ----
                for tk in range(8):
                    for hh, h in enumerate(heads):
                        sA = psp.tile(
                            [128, 1024], FP, name=f"sA_{h}_{tk}", tag="s", bufs=2
                        )
                        for sh in range(2):
                            nc.tensor.matmul(
                                sA[:, sh * 512 : (sh + 1) * 512],
                                (
                                    qT_s[dc][
                                        hh * 64 : hh * 64 + 64,
                                        tk * 128 : (tk + 1) * 128,
                                    ]
                                ),
                                (
                                    kT_s[dc][
                                        hh * 64 : hh * 64 + 64,
                                        sh * 512 : (sh + 1) * 512,
                                    ]
                                ),
                                start=True,
                                stop=True,
                            )
                        ex = expAp.tile([128, 1024], FP, name=f"ex_{h}_{tk}", tag="expA")
                        nc.scalar.activation(ex[:], sA[:], AF.Exp, scale=SCALE)
                        wA = wAp.tile([128, 1024], FP, name=f"wA_{h}_{tk}", tag="wA")
                        sums = smallp.tile(
                            [128, 1], FP, name=f"sums_{h}_{tk}", tag="sums", bufs=3
                        )
                        nc.vector.scalar_tensor_tensor(
                            out=wA[:],
                            in0=ex[:],
                            scalar=1.0,
                            in1=m01A_s[:, tk, :],
                            op0=ALU.mult,
                            op1=ALU.mult,
                            accum_out=sums[:],
                        )
                        nc.vector.reciprocal(r_h[h][:, tk : tk + 1], sums[:])
                        nc.vector.tensor_scalar_mul(
                            wA[:], wA[:], r_h[h][:, tk : tk + 1]
                        )
                        nc.sync.dma_start(
                            out=wout[h % HPC, tk * 128 : (tk + 1) * 128, :],
                            in_=wA[:],
                        )

                # ---- merged-norm: merged^T = AV * bcast(r) ----
                # Rt [128 t, 64(h0)|64(h1)] = r per-partition bcast; PE
                # transpose -> R_ps[:, tk-block] = [r_h0 rows; r_h1 rows].
                for tc in range(2):
                    R_ps = psp.tile(
                        [128, 512], FP, name=f"R_{dc}_{tc}", tag="s", bufs=2
                    )
                    for q in range(4):
                        tkk = tc * 4 + q
                        Rt = smallp.tile(
                            [128, 128], FP, name=f"Rt_{dc}_{tkk}", tag="Rt", bufs=2
                        )
                        for hh, h in enumerate(heads):
                            nc.vector.tensor_scalar_mul(
                                Rt[:, hh * 64 : hh * 64 + 64],
                                ones64[:],
                                r_h[h][:, tkk : tkk + 1],
                            )
                        nc.tensor.transpose(
                            R_ps[:, q * 128 : (q + 1) * 128], Rt[:], ident[:]
                        )
                    R_sb = smallp.tile(
                        [128, 512], FP, name=f"Rsb_{dc}_{tc}", tag="Rsb", bufs=2
                    )
                    nc.scalar.copy(R_sb[:], R_ps[:])
                    for hh, h in enumerate(heads):
                        nc.vector.tensor_tensor(
                            merged_s[dc][
                                hh * 64 : hh * 64 + 64, tc * 512 : (tc + 1) * 512
                            ],
                            av_ps[(hh, tc)][:],
                            R_sb[hh * 64 : hh * 64 + 64, :],
                            ALU.mult,
                        )

            # ================= Phase 3: O^T projection =================
            for jc in range(8):
                wot = wosp.tile([128, 4, 128], FR, name=f"wot_{jc}", tag="wo")
                nc.sync.dma_start(
                    out=wot[:],
                    in_=woT[:, jc * 128 : (jc + 1) * 128].rearrange(
                        "(c p) j -> p c j", p=128
                    ),
                )
                for tc in range(2):
                    o_ps = psp.tile(
                        [128, 512], FP, name=f"o_{jc}_{tc}", tag="b1", bufs=4
                    )
                    for dc in range(4):
                        nc.tensor.matmul(
                            o_ps[:],
                            (wot[:, dc, :]),
                            (merged_s[dc][:, tc * 512 : (tc + 1) * 512]),
                            start=(dc == 0),
                            stop=(dc == 3),
                        )
                    o_sb = osbp.tile([128, 512], FP, name=f"osb_{jc}_{tc}", tag="osb")
                    nc.vector.tensor_copy(o_sb[:], o_ps[:])
                    nc.sync.dma_start(
                        out=oout[
                            jc * 128 : (jc + 1) * 128, tc * 512 : (tc + 1) * 512
                        ],
                        in_=o_sb[:],
                    )

    _split_multi_waits(nc)
    return nc


_NC_CACHE = None


def _get_nc():
    global _NC_CACHE
    if _NC_CACHE is None:
        _NC_CACHE = build_nc()
    return _NC_CACHE


def make_in_maps(x, context, mask, W_Q, W_K, W_V, W_O):
    import ml_dtypes

    bf16 = ml_dtypes.bfloat16
    x = np.asarray(x, dtype=np.float32)
    context = np.asarray(context, dtype=np.float32)
    mask = np.asarray(mask)
    W_Q = np.asarray(W_Q, dtype=np.float32)
    W_K = np.asarray(W_K, dtype=np.float32)
    W_V = np.asarray(W_V, dtype=np.float32)
    W_O = np.asarray(W_O, dtype=np.float32)

    in_maps = []
    for c in range(N_CORES):
        b, hh = c // 2, c % 2
        rows = slice(hh * D, (hh + 1) * D)
        mb = mask[b, 0].astype(np.float32)  # [T, S], 1.0 = masked
        in_maps.append(
            {
                "xT": np.ascontiguousarray(x[b].T),
                "cT": np.ascontiguousarray(context[b].T),
                "wqk": np.ascontiguousarray(
                    np.concatenate([W_Q[rows].T, W_K[rows].T], axis=1)
                ),
                "wv": np.ascontiguousarray(W_V[rows].T),
                "woT": np.ascontiguousarray(W_O[:, rows].T),
                "m01A": (1.0 - mb).astype(bf16),
                "mB": np.ascontiguousarray((NEG * mb).T).astype(bf16),
            }
        )
    return in_maps


def assemble(results):
    """results: list of 8 dicts with 'wout' [8,T,S] and 'oout' [EMB,T]."""
    weights = np.empty((B, H, T, SRC), dtype=np.float32)
    out = np.empty((B, T, EMB), dtype=np.float32)
    for b in range(B):
        c0, c1 = 2 * b, 2 * b + 1
        weights[b, :HPC] = results[c0]["wout"]
        weights[b, HPC:] = results[c1]["wout"]
        out[b] = (results[c0]["oout"] + results[c1]["oout"]).T
    return out, weights


def kernel(x, context, mask, W_Q, W_K, W_V, W_O):
    nc = _get_nc()
    in_maps = make_in_maps(x, context, mask, W_Q, W_K, W_V, W_O)
    res = run_bass_kernel_spmd(nc, in_maps, core_ids=list(range(N_CORES)))
    return assemble(res.results)
